# revision 1
# baseline (speedup 1.0000x reference)
"""Trainium2 Bass kernel for BilinearPairedLayer (fp16 dataflow).

Math (reference):
  h = relu(x @ W_lin + b_lin)                      # [B, N, 32]
  v = concat(shift(h,-1), h, shift(h,+1))          # [B, N, 96]
  out[b,i,j,o] = v[b,i] @ W_bil[o] @ v[b,j] + b_bil[o]   # [B, N, N, 8]

Kernel strategy (8 cores, shard over output column dim j; identical NEFF,
per-core j-window via partition_id dynamic slices):

  The kernel is output-DMA bound (~23.3us/core fp16 on the modeled 360GB/s
  serialized DMA device; fp32 would be ~47us). The error metric
  (max|err|/max|expected| < 2e-2) has ~30x headroom over fp16, so the
  whole dataflow is fp16 (PSUM accumulates fp32): host supplies x
  transposed/augmented fp16 [65, B*N] (no PE transposes), fp16
  weights/staging/output (host widens to fp32). Measured rel err ~7e-4.

  vT layout: one [97, B*1026] tile; per-b blocks of 1026 columns with
  zero pad columns at both ends so the shifted copies propagate exact
  edge zeros. The v-component row order is [h, shift(-1)h, shift(+1)h]
  (h at partitions 0:32) with both contraction axes of W_bil permuted to
  match on the host: dynamic slices must be partition-0-based with pure
  reg starts (reg+const ds() starts and partition-offset ds() sources
  miscompute offsets in the AP lowering). Row 96 = ones, so the bilinear
  bias rides in the mains matmul via u row 96 = b_bil (tiny DMA).

  Per b: 2 h-matmuls -> relu drains into vT rows 0:32; the core's
  j-window [96,128] for u comes from three dynamic window-copies of the
  h rows (static col offsets d=0,1,2 emulate the shifts), so u depends
  only on the relus; 8 static matmuls + 2 merged permuted-AP drains
  build u[h,(j,o)]; 4 half-shift copies build vT rows 32:96 in parallel
  (they gate only the mains); mains: out[i,(j,o)] = vT.T @ u, 2 matmuls
  per 128-row i-chunk into a 2-bank PSUM tile, 1 drain, 1 DMA per chunk
  on SP.

  Scheduling notes:
   - GPSIMD/Pool cannot access PSUM on TRN2, so PSUM->SBUF drains go to
     DVE+ACT only (greedy cost-weighted); Pool takes all SBUF-side work
     (shift copies at fp16, window copies, memsets).
   - prep/u for b1..3 is frontloaded into explicit (mains-b, chunk)
     slots: h-matmul units go early (their x data has landed, so the
     relu never head-of-line-blocks a ready mains drain in the in-order
     engine queues), u-group drains spread late.
   - junk matmuls keep PE's p-state ramp alive until x_b0 lands (the
     cost model evaluates the ramp at dispatch); mains chunk drains
     strictly alternate DVE/ACT by chunk parity so consecutive output
     DMAs never serialize on one engine.
   - cost-model estimate: ~38.4us/core vs ~65.8us for the fp32r
     baseline; pure fp16 output-DMA floor is ~25us.
"""

import os
import numpy as np
from contextlib import ExitStack

B, N, NIN, NH, NOUT = 4, 1024, 64, 32, 8
H = 3 * NH  # 96
NCORES = 8
NJ = N // NCORES  # 128 output columns per core
NA = NIN + 1  # 65: x augmented with ones row
BW = N + 2  # 1026: padded block width in vT_all

_CACHE = {}

# output DMA groups per b (i-chunk spans): small first/last for latency
_G = os.environ.get("K_GROUPS", "8x1")
if _G == "8x1":
    DMA_GROUPS = [(i, i + 1) for i in range(8)]
if True and _G == "8x1":
    pass
if _G != "8x1":
    DMA_GROUPS, _c = [], 0
    for ch in _G:
        DMA_GROUPS.append((_c, _c + int(ch)))
        _c += int(ch)
    assert _c == 8, _G
_G0 = os.environ.get("K_GROUPS0", _G)
if _G0 == "8x1":
    DMA_GROUPS0 = [(i, i + 1) for i in range(8)]
else:
    DMA_GROUPS0, _c = [], 0
    for ch in _G0:
        DMA_GROUPS0.append((_c, _c + int(ch)))
        _c += int(ch)
    assert _c == 8, _G0
NJUNK = int(os.environ.get("K_NJUNK", "4"))
NOFORCE = os.environ.get("K_NOFORCE", "1") == "1"
F0 = os.environ.get("K_F0", "0") == "1"
UWINDVE = os.environ.get("K_UWINDVE", "1") == "1"
UACC = os.environ.get("K_UACC", "0") == "1"
NOLEDGER = os.environ.get("K_NOLEDGER", "1") == "1"
FRONTLOAD = os.environ.get("K_FRONT", "1") == "1"
FRONT_DENS = float(os.environ.get("K_FDENS", "1.2"))
FRONT_OFF = float(os.environ.get("K_FOFF", "0"))
FMAP = os.environ.get("K_FMAP", "1") == "1"
HEADSPLIT = os.environ.get("K_HEADSPLIT", "1") == "1"
TAILSPLIT = os.environ.get("K_TAILSPLIT", "0") == "1"
NOBRIDGE = os.environ.get("K_NOBRIDGE", "1") == "1"
ONESPOOL = os.environ.get("K_ONESPOOL", "0") == "1"
HALF_DRAINS = os.environ.get("K_HALF", "0") == "1"
FINE_HOOKS = os.environ.get("K_FINE", "0") == "1"
UWSPREAD = os.environ.get("K_UWSPREAD", "0") == "1"
HSWAP = os.environ.get("K_HSWAP", "1") == "1"
RSWAP = os.environ.get("K_RSWAP", "0") == "1"
USWAP = os.environ.get("K_USWAP", "1") == "1"
SSEL = os.environ.get("K_SSEL", "ap")
ALT = os.environ.get("K_ALT", "1") == "1"
ALTP = int(os.environ.get("K_ALTP", "0"))
OPP = os.environ.get("K_OPP", "0") == "1"
XMERGE = os.environ.get("K_XMERGE", "0") == "1"
PDMA = os.environ.get("K_PDMA", "0") == "1"
HSPLITN = int(os.environ.get("K_HSPLITN", "0"))
USPLITJ = os.environ.get("K_USPLITJ", "1") == "1"
UJSWAP = os.environ.get("K_UJSWAP", "1") == "1"
UJALL = os.environ.get("K_UJALL", "0") == "1"


class _Assigner:
    """Greedy engine picker: each op goes to the engine whose accumulated
    busy-time stays lowest. Costs are cost-model estimates in ns."""

    def __init__(self, nc, mybir):
        self.nc = nc
        self.mybir = mybir
        self.busy = {"dve": 0.0, "act": 0.0, "pool": 0.0}
        self.tag = None

    def note(self, eng, cost):
        if not NOLEDGER:
            self.busy[eng] += cost

    def pick(self, costs, force=None):
        if force is not None:
            eng = force
            self.busy[eng] += costs.get(eng, 800.0)
            return eng
        eng = min(costs, key=lambda e: self.busy[e] + costs[e])
        self.busy[eng] += costs[eng]
        return eng

    def copy(self, out, in_, costs, force=None):
        eng = self.pick(costs, force)
        if eng == "dve":
            i = self.nc.vector.tensor_copy(out, in_)
        elif eng == "act":
            i = self.nc.scalar.activation(
                out, in_, func=self.mybir.ActivationFunctionType.Copy
            )
        else:
            i = self.nc.gpsimd.tensor_copy(out, in_)
        if self.tag:
            i.annotate(self.tag)
        return i

    def relu(self, out, in_, costs, force=None):
        eng = self.pick(costs, force)
        if eng == "dve":
            i = self.nc.vector.tensor_scalar_max(out, in_, 0.0)
        elif eng == "act":
            i = self.nc.scalar.activation(
                out, in_, func=self.mybir.ActivationFunctionType.Relu
            )
        else:
            i = self.nc.gpsimd.tensor_scalar_max(out, in_, 0.0)
        if self.tag:
            i.annotate(self.tag)
        return i


def _build_nc(use_f32r: bool = True):
    import concourse.bass as bass
    import concourse.tile as tile
    from concourse import bacc, mybir

    f32 = mybir.dt.float32
    f16 = mybir.dt.float16

    nc = bacc.Bacc(
        "TRN2", target_bir_lowering=False, debug=False, num_devices=NCORES
    )

    xT_d = nc.dram_tensor("xT", [NA, B * N], f16, kind="ExternalInput").ap()
    wa_d = nc.dram_tensor("W_aug", [NA, NH], f16, kind="ExternalInput").ap()
    # W_bilT[g, o, h] = W_bil[o, h, g]
    wb_d = nc.dram_tensor("W_bilT", [H, NOUT, H], f16, kind="ExternalInput").ap()
    bu_d = nc.dram_tensor("bias_u", [1, B, NJ, NOUT], f16, kind="ExternalInput").ap()
    on_d = nc.dram_tensor("ones_row", [1, B * BW], f16, kind="ExternalInput").ap()
    out_d = [
        nc.dram_tensor(f"out_{b}", [N, NJ, NOUT], f16, kind="ExternalOutput").ap()
        for b in range(B)
    ]

    CopyF = mybir.ActivationFunctionType.Copy

    with ExitStack() as ctx:
        tc = ctx.enter_context(tile.TileContext(nc))
        consts = ctx.enter_context(tc.tile_pool(name="consts", bufs=1))
        stage = ctx.enter_context(tc.tile_pool(name="stage", bufs=10))
        ps_s = ctx.enter_context(tc.tile_pool(name="ps_s", bufs=2, space="PSUM"))
        ps_m = ctx.enter_context(
            tc.tile_pool(name="ps_m", bufs=6 if HALF_DRAINS else 3, space="PSUM")
        )

        asg = _Assigner(nc, mybir)
        # measured drain costs (ns) for the greedy engine assignment
        # PSUM sources: GPSIMD/Pool cannot access PSUM on TRN2 -> DVE/ACT only
        C_FULL = {"dve": 1240.0, "act": 1030.0}  # [128,1024] psum drain
        C_HALF = {"dve": 700.0, "act": 600.0}  # [128,512] psum drain
        C_H = {"dve": 700.0, "act": 600.0}  # relu [32,512] psum drain
        C_U = {"dve": 700.0, "act": 600.0}  # u merged [96,512] psum drain
        # SBUF->SBUF: Pool allowed (and preferred, it has slack)
        C_SHIFT = {"dve": 360.0, "act": 890.0, "pool": 807.0}  # f16 2x on DVE
        C_UWIN = {"dve": 330.0, "pool": 273.0}

        # --- input DMAs, all on SP, criticality order (outputs follow)
        xTs = consts.tile([NA, B * N], f16, tag="xTs")
        wa_sb = consts.tile([NA, NH], f16, tag="wa")
        wb_sb = consts.tile([H, NOUT, H], f16, tag="wb")
        u_all = consts.tile([H + 1, B, NJ, NOUT], f16, tag="u_all")
        vT = consts.tile([H + 1, B * BW], f16, tag="vT")
        with nc.allow_non_contiguous_dma(reason="per-b column slices"):
            nc.sync.dma_start(out=xTs[:, 0:N], in_=xT_d[:, 0:N]).annotate("xdma0")
        nc.scalar.dma_start(out=wa_sb, in_=wa_d).annotate("wa")
        for b in range(B):
            nc.gpsimd.memset(vT[96:97, b * BW : (b + 1) * BW], 1.0)
        if UACC:
            wb3 = consts.tile([32, 3, NOUT, H], f16, tag="wb3")
            with nc.allow_non_contiguous_dma(reason="g-block-major weight load"):
                nc.sync.dma_start(
                    out=wb3, in_=wb_d.rearrange("(k p) o h -> p k o h", p=32)
                ).annotate("wb")
        else:
            nc.sync.dma_start(out=wb_sb, in_=wb_d).annotate("wb")
        nc.sync.dma_start(out=u_all[96:97, :, :, :], in_=bu_d).annotate("bias")
        with nc.allow_non_contiguous_dma(reason="per-b column slices"):
            if XMERGE:
                nc.sync.dma_start(
                    out=xTs[:, N : B * N], in_=xT_d[:, N : B * N]
                ).annotate("xdma123")
            else:
                for b in range(1, B):
                    nc.sync.dma_start(
                        out=xTs[:, b * N : (b + 1) * N],
                        in_=xT_d[:, b * N : (b + 1) * N],
                    ).annotate(f"xdma{b}")

        # zero pad columns (cols 0 and 1025 of each block's h rows)
        for b in range(B):
            nc.gpsimd.memset(vT[0:32, b * BW : b * BW + 1], 0.0)
            nc.gpsimd.memset(vT[0:32, b * BW + BW - 1 : b * BW + BW], 0.0)

        # --- PE p-state warmup: junk matmuls chained under the input DMAs
        junk16 = consts.tile([128, 512], f16, tag="junk16")
        nc.vector.memset(junk16[:], 1.0)
        # pre-warm ACT's function table off the critical path
        act_warm = consts.tile([1, 1], f32, tag="act_warm")
        nc.scalar.activation(act_warm, junk16[0:1, 0:1], func=CopyF)
        nc.scalar.activation(
            act_warm, junk16[0:1, 0:1], func=mybir.ActivationFunctionType.Relu
        )
        asg.note("act", 170.0)
        for k in range(NJUNK):
            jp = ps_m.tile([32, 512], f32, tag="ps", name="jp")
            nc.tensor.matmul(
                jp, lhsT=junk16[:, 0:32], rhs=junk16[:], start=True, stop=True
            ).annotate(f"junk{k}")

        jlo = nc.vector.partition_id() * NJ
        jlo_pool = nc.gpsimd.partition_id() * NJ
        if UACC:
            uwin = [
                consts.tile([32, NJ + 2], f16, tag=f"uw{b}", name=f"uw{b}")
                for b in range(B)
            ]
        else:
            uwin = [
                consts.tile([H, NJ], f16, tag=f"uw{b}", name=f"uw{b}")
                for b in range(B)
            ]

        def prep_h(b, k, first=False, rforce=None):
            base = b * BW
            ph = ps_s.tile([NH, 512], f32, tag="ps", name="ph")
            nc.tensor.matmul(
                ph,
                lhsT=wa_sb[:],
                rhs=xTs[:, b * N + k * 512 : b * N + (k + 1) * 512],
                start=True,
                stop=True,
            ).annotate(f"h_mm{b}.{k}")
            asg.tag = f"relu{b}.{k}"
            force = (("dve", "act") if RSWAP else ("act", "dve"))[k] if first else rforce
            asg.relu(
                vT[0:32, base + 1 + k * 512 : base + 1 + (k + 1) * 512],
                ph,
                C_H,
                force=force,
            )

        def prep_shift(b, half, first=False):
            # 2 half-shift copies depending on relu half `half`; zeros
            # propagate from the pad columns; these gate only the mains
            base = b * BW
            hb = vT[0:32, :]
            asg.tag = f"shift{b}"
            fs = (
                {"ap": ("act", "pool"), "pa": ("pool", "act"), "pp": ("pool", "pool"),
                 "aa": ("act", "act")}[SSEL]
                if first
                else ("pool", "pool")
            )
            if half == 0:
                asg.copy(
                    vT[32:64, base + 1 : base + 514], hb[:, base : base + 513],
                    C_SHIFT, force=fs[0],
                )
                asg.copy(
                    vT[64:96, base : base + 512], hb[:, base + 1 : base + 513],
                    C_SHIFT, force=fs[1],
                )
            else:
                asg.copy(
                    vT[32:64, base + 514 : base + BW],
                    hb[:, base + 513 : base + BW - 1],
                    C_SHIFT, force=fs[0],
                )
                asg.copy(
                    vT[64:96, base + 512 : base + BW - 1],
                    hb[:, base + 513 : base + BW],
                    C_SHIFT, force=fs[1],
                )

        def prep(b, first=False):
            prep_h(b, 0, first)
            prep_h(b, 1, first)
            prep_shift(b, 0, first)
            prep_shift(b, 1, first)

        def prep0():
            # b0: uwin copies emitted before the shifts so DVE runs them
            # right after relu (shifts go to ACT/Pool in parallel)
            prep_h(0, 0, True)
            prep_h(0, 1, True)
            uwin_b(0)
            prep_shift(0, 0, True)
            prep_shift(0, 1, True)

        def uwin_b(b):
            # the core's j-window, straight from the relu'd h rows: depends
            # only on the relus (plus pad memsets).
            base = b * BW
            if UACC:
                # one [32,130]-wide dynamic window; the three shift variants
                # become static column slices consumed by accumulating u mms
                winw = vT[0:32, base : base + BW]
                if b == 0:
                    nc.vector.tensor_copy(
                        uwin[b][:], winw[:, bass.ds(jlo, NJ + 2)]
                    ).annotate(f"uwinw{b}")
                else:
                    nc.gpsimd.tensor_copy(
                        uwin[b][:], winw[:, bass.ds(jlo_pool, NJ + 2)]
                    ).annotate(f"uwinw{b}")
                return
            DSTROW = {1: 0, 0: 32, 2: 64}  # matches host-side W_bil perm
            for d in range(3):
                r = DSTROW[d]
                dst = uwin[b][r : r + 32, :]
                # static window [base+d, base+d+N); dynamic part stays a pure
                # ds(partition_id*NJ, NJ) on a partition-0-based source (both
                # reg+const ds() starts and partition-offset ds() sources
                # miscompute offsets)
                win = vT[0:32, base + d : base + d + N]
                f0 = ("pool", "dve", "pool")[d] if UWSPREAD else "dve"
                eng = asg.pick(C_UWIN, force=f0 if b == 0 else None)
                if eng == "pool":
                    nc.gpsimd.tensor_copy(
                        dst, win[:, bass.ds(jlo_pool, NJ)]
                    ).annotate(f"uwin{b}.{d}")
                else:
                    nc.vector.tensor_copy(
                        dst, win[:, bass.ds(jlo, NJ)]
                    ).annotate(f"uwin{b}.{d}")

        def u_g(b, og, first=False, uforce=None):
            # u[h, (j,o)] = sum_g W_bil[o,h,g] v[j,g]; 4 o's per PSUM bank,
            # merged permuted-AP drain
            pu = ps_s.tile([H, 4, NJ], f32, tag="ps", name="pu")
            for oo in range(4):
                if UACC:
                    # accumulate the three shift components; rhs are STATIC
                    # slices of the wide window (g-order [h, s-1, s+1] pairs
                    # with window cols 1:129 / 0:128 / 2:130)
                    for k, (g0, c0) in enumerate(((0, 1), (32, 0), (64, 2))):
                        nc.tensor.matmul(
                            pu[:, oo, :],
                            lhsT=wb3[:, g0 // 32, og * 4 + oo, :],
                            rhs=uwin[b][:, c0 : c0 + NJ],
                            start=(k == 0),
                            stop=(k == 2),
                        ).annotate(f"u_mm{b}.{og}.{oo}.{k}")
                    continue
                nc.tensor.matmul(
                    pu[:, oo, :],
                    lhsT=wb_sb[:, og * 4 + oo, :],
                    rhs=uwin[b][:],
                    start=True,
                    stop=True,
                ).annotate(f"u_mm{b}.{og}.{oo}")
            asg.tag = f"u_dr{b}.{og}"
            fu = (("dve", "act") if USWAP else ("act", "dve"))[og] if first else uforce
            asg.copy(
                u_all[0:H, b, :, og * 4 : (og + 1) * 4],
                pu.rearrange("p o j -> p j o"),
                C_U,
                force=fu,
            )

        def u_bj(b):
            # j-half-split u for the startup batch: each drain covers exactly
            # one mains matmul's rhs slice, so m_mm(b).0.jh waits ONE drain
            for jh in range(2):
                pu = ps_s.tile([H, NOUT, 64], f32, tag="ps", name="puj")
                for oo in range(NOUT):
                    nc.tensor.matmul(
                        pu[:, oo, :],
                        lhsT=wb_sb[:, oo, :],
                        rhs=uwin[b][:, jh * 64 : (jh + 1) * 64],
                        start=True,
                        stop=True,
                    ).annotate(f"u_mmj{b}.{jh}.{oo}")
                asg.tag = f"u_drj{b}.{jh}"
                asg.copy(
                    u_all[0:H, b, jh * 64 : (jh + 1) * 64, :],
                    pu.rearrange("p o j -> p j o"),
                    C_U,
                    force=(("act", "dve") if UJSWAP else ("dve", "act"))[jh],
                )

        def u_b(b, first=False):
            if USPLITJ and (first or UJALL):
                u_bj(b)
                return
            u_g(b, 0, first)
            u_g(b, 1, first)

        def mains(b, hooks):
            base = b * BW
            groups = DMA_GROUPS0 if b == 0 else DMA_GROUPS
            for g, (c0, c1) in enumerate(groups):
                nch = c1 - c0
                ot = stage.tile([128, nch, NJ * NOUT], f16, tag="ot", name="ot")
                for d in range(nch):
                    ic = c0 + d
                    if ic in hooks:
                        hooks[ic]()
                    if HALF_DRAINS:
                        for jh in range(2):
                            pmh = ps_m.tile([128, 512], f32, tag="ps", name="pmh")
                            nc.tensor.matmul(
                                pmh,
                                lhsT=vT[:, base + 1 + ic * 128 : base + 1 + (ic + 1) * 128],
                                rhs=u_all[:, b, jh * 64 : (jh + 1) * 64, :],
                                start=True,
                                stop=True,
                            ).annotate(f"m_mm{b}.{ic}.{jh}")
                            asg.tag = f"m_dr{b}.{ic}.{jh}"
                            asg.copy(
                                ot[:, d, jh * 512 : (jh + 1) * 512],
                                pmh,
                                C_HALF,
                                force=("act", "dve")[jh] if b == 0 and ic < 2 else None,
                            )
                        continue_drain = True
                    else:
                        pm = ps_m.tile([128, NJ * NOUT], f32, tag="ps", name="pm")
                        for jh in range(2):
                            nc.tensor.matmul(
                                pm[:, jh * 512 : (jh + 1) * 512],
                                lhsT=vT[:, base + 1 + ic * 128 : base + 1 + (ic + 1) * 128],
                                rhs=u_all[:, b, jh * 64 : (jh + 1) * 64, :],
                                start=True,
                                stop=True,
                            ).annotate(f"m_mm{b}.{ic}.{jh}")
                        if (b == 0 and ic < 2) or (
                            TAILSPLIT and b == B - 1 and ic == 7
                        ):
                            for jh in range(2):
                                asg.tag = f"m_dr{b}.{ic}.{jh}"
                                asg.copy(
                                    ot[:, d, jh * 512 : (jh + 1) * 512],
                                    pm[:, jh * 512 : (jh + 1) * 512],
                                    C_HALF,
                                    force=(("dve", "act") if HSWAP else ("act", "dve"))[jh],
                                )
                        else:
                            asg.tag = f"m_dr{b}.{ic}"
                            asg.copy(
                                ot[:, d, :], pm, C_FULL,
                                force=("dve", "act")[(ic + ALTP) % 2] if ALT else None,
                            )
                sp_first = HEADSPLIT and b == 0 and g <= HSPLITN
                sp_last = TAILSPLIT and b == B - 1 and g == len(groups) - 1
                if (sp_first or sp_last) and c1 - c0 == 1:
                    odh = out_d[b][c0 * 128 : c1 * 128, :, :].rearrange(
                        "p j o -> p (j o)"
                    )
                    with nc.allow_non_contiguous_dma(reason="edge half store"):
                        for jh in range(2):
                            nc.sync.dma_start(
                                out=odh[:, jh * 512 : (jh + 1) * 512],
                                in_=ot[:, 0, jh * 512 : (jh + 1) * 512],
                            ).annotate(f"odma{b}.{g}.{jh}")
                else:
                    od_g = out_d[b][c0 * 128 : c1 * 128, :, :].rearrange(
                        "(d p) j o -> p d (j o)", p=128
                    )
                    with nc.allow_non_contiguous_dma(
                        reason="grouped row-chunk store"
                    ):
                        if PDMA and c0 % 4 == 3:
                            # SWDGE path: bypasses the HWDGE device (86%
                            # utilized mid-stream); Pool has headroom
                            nc.gpsimd.dma_start(
                                out=od_g, in_=ot
                            ).annotate(f"odma{b}.{g}")
                        else:
                            nc.sync.dma_start(
                                out=od_g, in_=ot
                            ).annotate(f"odma{b}.{g}")

        def junk_mm(tag):
            jp = ps_m.tile([32, 512], f32, tag="ps", name="jp")
            nc.tensor.matmul(
                jp, lhsT=junk16[:, 0:32], rhs=junk16[:], start=True, stop=True
            ).annotate(tag)

        prep0()
        if not NOBRIDGE:
            for k in range(3):
                junk_mm(f"junkh{k}")
        u_b(0, first=True)
        if not NOBRIDGE:
            for k in range(2):
                junk_mm(f"junku{k}")
        if FRONTLOAD:
            # all prep/u work for b1..3 as a unit queue, mapped onto the
            # low-DMA-density early chunks; the dense tail runs insertion-free
            units = []
            for bb in range(1, B):
                units += [
                    (bb, lambda bb=bb: prep_h(bb, 0)),
                    (bb, lambda bb=bb: prep_h(bb, 1)),
                    (bb, lambda bb=bb: (prep_shift(bb, 0), prep_shift(bb, 1))),
                    (bb, lambda bb=bb: uwin_b(bb)),
                    (bb, lambda bb=bb: u_g(bb, 0)),
                    (bb, lambda bb=bb: u_g(bb, 1)),
                ]
            # slots: (mains-b, chunk) positions across the first three
            # mains blocks (per-unit clamp keeps deps valid)
            slots = (
                [(0, c) for c in range(1, 8)]
                + [(1, c) for c in range(0, 8)]
                + [(2, c) for c in range(0, 8)]
            )
            per = {}
            if FMAP:
                # explicit per-unit slots: 6 units per b in emission order
                # (h0, h1, shifts, uwin, u_g0, u_g1)
                table = {
                    1: [(0, 1), (0, 2), (0, 4), (0, 5), (0, 7), (1, 0)],
                    2: [(0, 3), (0, 5), (1, 1), (1, 2), (1, 4), (1, 6)],
                    3: [(1, 2), (1, 4), (1, 7), (2, 0), (2, 2), (2, 4)],
                }
                cnt = {}
                for bb, u in units:
                    k = cnt.get(bb, 0)
                    cnt[bb] = k + 1
                    s = table[bb][k]
                    if OPP and k in (0, 1, 4, 5):
                        # PSUM op inserted at slot (sb, c): the chunk drain
                        # there goes to ("dve","act")[c%2]; put this op on
                        # the opposite engine so it never delays the stream
                        opp = ("act", "dve")[s[1] % 2]
                        if k in (0, 1):
                            u = (lambda bb=bb, kk=k, o=opp: prep_h(bb, kk, rforce=o))
                        else:
                            u = (lambda bb=bb, og=k - 4, o=opp: u_g(bb, og, uforce=o))
                    per.setdefault(s, []).append(u)
            else:
                for i, (bb, u) in enumerate(units):
                    idx = int(i * len(slots) / len(units) * FRONT_DENS + FRONT_OFF)
                    # unit for batch bb must be emitted before mains(bb) starts
                    last_ok = max(k for k, (sb, _c) in enumerate(slots) if sb < bb)
                    s = slots[min(idx, last_ok)]
                    per.setdefault(s, []).append(u)
            for b in range(B):
                hooks = {
                    c: (lambda fns=fns: [f() for f in fns])
                    for (sb, c), fns in per.items()
                    if sb == b
                }
                mains(b, hooks)
        else:
            for b in range(B):
                hooks = {}
                if b + 1 < B:
                    bb = b + 1
                    if FINE_HOOKS:
                        hooks = {
                            1: (lambda bb=bb: prep_h(bb, 0)),
                            2: (lambda bb=bb: prep_h(bb, 1)),
                            3: (lambda bb=bb: (prep_shift(bb, 0), prep_shift(bb, 1))),
                            4: (lambda bb=bb: uwin_b(bb)),
                            5: (lambda bb=bb: u_g(bb, 0)),
                            6: (lambda bb=bb: u_g(bb, 1)),
                        }
                    else:
                        hooks = {
                            0: (lambda bb=bb: (prep(bb), uwin_b(bb))),
                            3: (lambda bb=bb: u_b(bb)),
                        }
                mains(b, hooks)

    nc.compile()
    return nc


def _prep_inputs(x, W_lin, b_lin, W_bil, b_bil):
    x = np.asarray(x, np.float32)
    xT = np.empty((NA, B * N), np.float16)
    xT[:NIN] = x.transpose(2, 0, 1).reshape(NIN, B * N)
    xT[NIN] = 1.0

    W_aug = np.concatenate(
        [np.asarray(W_lin, np.float32), np.asarray(b_lin, np.float32)[None, :]],
        axis=0,
    ).astype(np.float16)  # [65, 32]
    # v-component order in the kernel is [h, shift(-1)h, shift(+1)h] (h rows
    # live at partitions 0:32 so dynamic slices stay partition-0-based);
    # permute both contraction axes of W_bil to match
    P = np.r_[32:64, 0:32, 64:96]
    W_bilT = np.ascontiguousarray(
        np.asarray(W_bil, np.float32).transpose(2, 0, 1)[P][:, :, P]
    ).astype(np.float16)  # [g, o, h], permuted
    bias_u = np.broadcast_to(
        np.asarray(b_bil, np.float16)[None, None, None, :], (1, B, NJ, NOUT)
    ).copy()
    ones_row = np.ones((1, B * BW), np.float16)

    shared = {
        "xT": xT,
        "W_aug": W_aug,
        "W_bilT": W_bilT,
        "bias_u": bias_u,
        "ones_row": ones_row,
    }
    return [dict(shared) for _ in range(NCORES)]


def _run(inputs, trace=False, use_f32r=None):
    from concourse.bass_utils import run_bass_kernel_spmd

    key = "nc"
    if key not in _CACHE:
        _CACHE[key] = _build_nc()
    nc = _CACHE[key]

    in_maps = _prep_inputs(
        inputs["x"], inputs["W_lin"], inputs["b_lin"], inputs["W_bil"], inputs["b_bil"]
    )
    res = run_bass_kernel_spmd(nc, in_maps, core_ids=list(range(NCORES)), trace=trace)
    out = np.empty((B, N, N, NOUT), dtype=np.float32)
    for c, r in enumerate(res.results):
        for b in range(B):
            out[b, :, c * NJ : (c + 1) * NJ, :] = r[f"out_{b}"].astype(np.float32)
    return out, res


def kernel(**inputs):
    out, _ = _run(inputs, trace=False)
    return out

